# revision 11
# baseline (speedup 1.0000x reference)
"""ConvLSTM + FC head on 8 Trainium2 NeuronCores.

Reference computation (see problem): x [B=4, S=32, C=128, H=32, W=32],
ConvLSTM with HID=128, 3x3 SAME conv over concat(x_t, h), scanned over S;
then spatial mean -> relu(fc) -> two scalar heads -> (offset, angle),
each [B, S, 1].

Sharding: 8 cores = 4 batch elements x 2-way split of the H dimension
(rows 0..15 / 16..31).  Each step a core computes its 16 rows of the new
hidden state; the single-row halo of h needed by the 3x3 conv is exchanged
between the pair through a 2-rank AllGather.

Schedule (per step, all conv matmuls bf16 into fp32 PSUM):
  tensor:  hpart-interior(t) | hpart-boundary(t) | xpart(t+2)
  scalar:  4 gate activations, tanh(c) for boundary rows then interior
  vector:  boundary-row state update first -> snd row -> AllGather launch
           (hidden behind xpart(t+2) + hpart-interior(t+1)), then interior
           state update, pooled-sum reduce, ghost-row writes from the
           AllGather result.
The boundary rows {0,15} of the gates are accumulated last (stride-15
matmuls) so the interior work never waits on the halo exchange, and the
exchange result is only needed by the *boundary* matmuls of step t+1,
one full tensor block later.
"""

import numpy as np
import ml_dtypes

import concourse.bass as bass
from concourse import bacc
import concourse.mybir as mybir
import concourse.tile as tile
from concourse.bass_utils import run_bass_kernel_spmd

B, S, C, H, W = 4, 32, 128, 32, 32
HID = 128
NR = 16                  # own rows per core
BR, BC = NR + 2, W + 2   # buffered rows/cols (halo rows + zero-pad cols)
PAIRS = [[0, 1], [2, 3], [4, 5], [6, 7]]
F32 = mybir.dt.float32
BF16 = mybir.dt.bfloat16
NP_BF16 = ml_dtypes.bfloat16
AFT = mybir.ActivationFunctionType
ALU = mybir.AluOpType
AXT = mybir.AxisListType

_cache = {}

# boundary rows of the 16-row slab: {0, 15} via stride-15 slices
BSL = slice(0, NR, NR - 1)          # ps/cst rows {0,15}
ISL = slice(1, NR - 1)              # ps/cst rows 1..14


def _build(use_coll=True, n_steps=S):
    nc = bacc.Bacc("TRN2", target_bir_lowering=False, debug=False, num_devices=8)
    xs = nc.dram_tensor("xs", [S, C, BR, BC], BF16, kind="ExternalInput").ap()
    wx = nc.dram_tensor("wx", [C, 4, 9, HID], BF16, kind="ExternalInput").ap()
    wh = nc.dram_tensor("wh", [HID, 4, 9, HID], BF16, kind="ExternalInput").ap()
    cb = nc.dram_tensor("cb", [HID, 4], F32, kind="ExternalInput").ap()
    ih = nc.dram_tensor("ih", [HID, 1], F32, kind="ExternalInput").ap()
    ic = nc.dram_tensor("ic", [HID, 1], F32, kind="ExternalInput").ap()
    fcw = nc.dram_tensor("fcw", [HID, C], F32, kind="ExternalInput").ap()
    fcb = nc.dram_tensor("fcb", [C, 1], F32, kind="ExternalInput").ap()
    fhw = nc.dram_tensor("fhw", [C, 2], F32, kind="ExternalInput").ap()
    fhb = nc.dram_tensor("fhb", [2, 1], F32, kind="ExternalInput").ap()
    msk = nc.dram_tensor("msk", [128, 4], F32, kind="ExternalInput").ap()
    out = nc.dram_tensor("out", [2, S], F32, kind="ExternalOutput").ap()

    with tile.TileContext(nc) as tc:
        with (
            tc.tile_pool(name="consts", bufs=1) as consts,
            tc.tile_pool(name="xpool", bufs=3) as xpool,
            tc.tile_pool(name="work", bufs=2) as work,
            tc.tile_pool(name="state", bufs=1) as state,
            tc.tile_pool(name="psum", bufs=2, space="PSUM") as psum,
            tc.tile_pool(name="dram", bufs=2, space="DRAM") as dram,
        ):
            # ---- small constants + first x tiles first (unblock init + MMs)
            cb_sb = consts.tile([HID, 4], F32, name="cb_sb")
            nc.sync.dma_start(out=cb_sb[:], in_=cb)
            ih_sb = consts.tile([HID, 1], F32, name="ih_sb")
            nc.sync.dma_start(out=ih_sb[:], in_=ih)
            ic_sb = consts.tile([HID, 1], F32, name="ic_sb")
            nc.sync.dma_start(out=ic_sb[:], in_=ic)
            msk_sb = consts.tile([128, 4], F32, name="msk_sb")
            nc.sync.dma_start(out=msk_sb[:], in_=msk)
            xt = {}
            for t0 in range(2):
                xt[t0] = xpool.tile([C, BR, BC], BF16, tag="x", name=f"x_{t0}")
                nc.sync.dma_start(out=xt[t0][:], in_=xs[t0])
            wx_sb = consts.tile([C, 4, 9, HID], BF16, name="wx_sb")
            nc.sync.dma_start(out=wx_sb[:], in_=wx)
            wh_sb = consts.tile([HID, 4, 9, HID], BF16, name="wh_sb")
            nc.sync.dma_start(out=wh_sb[:], in_=wh)
            fcw_sb = consts.tile([HID, C], F32, name="fcw_sb")
            nc.sync.dma_start(out=fcw_sb[:], in_=fcw)
            fcb_sb = consts.tile([C, 1], F32, name="fcb_sb")
            nc.sync.dma_start(out=fcb_sb[:], in_=fcb)
            fhw_sb = consts.tile([C, 2], F32, name="fhw_sb")
            nc.sync.dma_start(out=fhw_sb[:], in_=fhw)
            fhb_sb = consts.tile([2, 1], F32, name="fhb_sb")
            nc.sync.dma_start(out=fhb_sb[:], in_=fhb)

            # ---- collective-path warmup: dummy AllGather + AllReduce so the
            # CC core's first-use queue setup (~10us) is off the critical path
            if use_coll:
                wrm = work.tile([HID, 1], F32, tag="wrm", name="wrm")
                nc.vector.memset(wrm[:], 0.0)
                wag_i = dram.tile([HID, 1], F32, tag="agin", name="wag_i")
                wag_o = dram.tile([2 * HID, 1], F32, tag="agout", name="wag_o")
                nc.gpsimd.dma_start(out=wag_i[:], in_=wrm[:])
                nc.gpsimd.collective_compute(
                    "AllGather", ALU.bypass, replica_groups=PAIRS,
                    ins=[wag_i[:].opt()], outs=[wag_o[:].opt()],
                )
                war_i = dram.tile([HID, 1], F32, tag="arin", name="war_i")
                war_o = dram.tile([HID, 1], F32, tag="arout", name="war_o")
                nc.gpsimd.dma_start(out=war_i[:], in_=wrm[:])
                nc.gpsimd.collective_compute(
                    "AllReduce", ALU.add, replica_groups=PAIRS,
                    ins=[war_i[:].opt()], outs=[war_o[:].opt()],
                )

            s0 = msk_sb[:, 0:1]
            s1 = msk_sb[:, 1:2]
            q0 = msk_sb[:, 2:3]
            q1 = msk_sb[:, 3:4]

            ihq0 = consts.tile([HID, 1], F32, name="ihq0")
            nc.vector.tensor_mul(ihq0[:], ih_sb[:], q0)
            ihq1 = consts.tile([HID, 1], F32, name="ihq1")
            nc.vector.tensor_mul(ihq1[:], ih_sb[:], q1)

            hsum = state.tile([HID, S], F32, name="hsum")

            # ---- persistent h buffers (even/odd steps); pad cols zeroed once
            hb = [
                state.tile([HID, BR, BC], BF16, name="h_even"),
                state.tile([HID, BR, BC], BF16, name="h_odd"),
            ]
            cst = state.tile([HID, NR, W], F32, name="cst")
            nc.vector.memset(cst[:], 0.0)
            # h_even holds h(0): interior = ih, ghost rows masked, pads zero
            nc.vector.memset(hb[0][:], 0.0)
            nc.vector.tensor_scalar_add(
                hb[0][:, 1 : NR + 1, 1 : W + 1], cst[:], ih_sb[:, 0:1]
            )
            nc.vector.tensor_scalar_add(
                hb[0][:, 0, 1 : W + 1], cst[:, 0, :], ihq0[:, 0:1]
            )
            nc.vector.tensor_scalar_add(
                hb[0][:, NR + 1, 1 : W + 1], cst[:, 0, :], ihq1[:, 0:1]
            )
            # h_odd: only the pad cols need to start (and stay) zero
            nc.vector.memset(hb[1][:, :, 0:1], 0.0)
            nc.vector.memset(hb[1][:, :, W + 1 : W + 2], 0.0)
            nc.vector.tensor_scalar_add(cst[:], cst[:], ic_sb[:, 0:1])

            def xpart(ps, x):
                for g in range(4):
                    for tap in range(9):
                        dy, dx = divmod(tap, 3)
                        nc.tensor.matmul(
                            ps[g][:],
                            wx_sb[:, g, tap, :],
                            x[:, dy : dy + NR, dx : dx + W],
                            start=(tap == 0),
                            stop=False,
                        )

            def hpart_int(ps, h):
                # output rows 1..14 <- input buffer rows (1+dy)..(14+dy)
                for g in range(4):
                    for tap in range(9):
                        dy, dx = divmod(tap, 3)
                        nc.tensor.matmul(
                            ps[g][:, ISL, :],
                            wh_sb[:, g, tap, :],
                            h[:, 1 + dy : NR - 1 + dy, dx : dx + W],
                            start=False,
                            stop=False,
                        )

            def hpart_bnd(ps, h):
                # output rows {0,15} <- input buffer rows {dy, 15+dy}
                for g in range(4):
                    for tap in range(9):
                        dy, dx = divmod(tap, 3)
                        nc.tensor.matmul(
                            ps[g][:, BSL, :],
                            wh_sb[:, g, tap, :],
                            h[:, dy : dy + NR : NR - 1, dx : dx + W],
                            start=False,
                            stop=(tap == 8),
                        )

            # ---- prologue: conv x-parts of the first two steps
            ps = {}
            for t0 in range(2):
                ps[t0] = [
                    psum.tile([HID, NR, W], F32, tag=f"ps{g}", name=f"ps{g}_{t0}")
                    for g in range(4)
                ]
                xpart(ps[t0], xt[t0])

            hcur = hb[0]
            for t in range(n_steps):
                if t + 2 < n_steps:
                    xt[t + 2] = xpool.tile([C, BR, BC], BF16, tag="x", name=f"x_{t+2}")
                    nc.sync.dma_start(out=xt[t + 2][:], in_=xs[t + 2])
                hn = hb[(t + 1) % 2]
                p = ps.pop(t)
                hpart_int(p, hcur)
                hpart_bnd(p, hcur)

                # ---- gate activations, ordered so the send-row chain can
                # launch ASAP: fg/ig/gg full, og boundary-only early, og
                # interior later
                fg = work.tile([HID, NR, W], F32, tag="fg", name=f"fg_{t}")
                nc.scalar.activation(fg[:], p[1][:], AFT.Sigmoid, bias=cb_sb[:, 1:2])
                ig = work.tile([HID, NR, W], F32, tag="ig", name=f"ig_{t}")
                nc.scalar.activation(ig[:], p[0][:], AFT.Sigmoid, bias=cb_sb[:, 0:1])
                gg = work.tile([HID, NR, W], F32, tag="gg", name=f"gg_{t}")
                nc.scalar.activation(gg[:], p[3][:], AFT.Tanh, bias=cb_sb[:, 3:4])
                ogb = work.tile([HID, 2, W], F32, tag="ogb", name=f"ogb_{t}")
                nc.scalar.activation(
                    ogb[:], p[2][:, BSL, :], AFT.Sigmoid, bias=cb_sb[:, 2:3]
                )

                # ---- boundary rows first: state update -> send row
                ub = work.tile([HID, 2, W], F32, tag="ub", name=f"ub_{t}")
                nc.vector.tensor_mul(ub[:], fg[:, BSL, :], cst[:, BSL, :])
                vb = work.tile([HID, 2, W], F32, tag="vb", name=f"vb_{t}")
                nc.vector.tensor_mul(vb[:], ig[:, BSL, :], gg[:, BSL, :])
                nc.vector.tensor_add(cst[:, BSL, :], ub[:], vb[:])
                tchb = work.tile([HID, 2, W], F32, tag="tchb", name=f"tchb_{t}")
                nc.scalar.activation(tchb[:], cst[:, BSL, :], AFT.Tanh)
                nc.vector.tensor_mul(
                    hn[:, 1 : NR + 1 : NR - 1, 1 : W + 1], ogb[:], tchb[:]
                )

                if t + 1 < n_steps:
                    # send row: top sends its row 16 (image row 15), bottom row 1
                    tmp = work.tile([HID, W], BF16, tag="tmp", name=f"tmp_{t}")
                    nc.vector.tensor_scalar_mul(tmp[:], hn[:, NR, 1 : W + 1], s0)
                    snd = work.tile([HID, W], BF16, tag="snd", name=f"snd_{t}")
                    nc.vector.scalar_tensor_tensor(
                        snd[:], hn[:, 1, 1 : W + 1], s1, tmp[:],
                        op0=ALU.mult, op1=ALU.add,
                    )
                    agin = dram.tile([HID, W], BF16, tag="agin", name=f"agin_{t}")
                    agout = dram.tile([2 * HID, W], BF16, tag="agout", name=f"agout_{t}")
                    if use_coll:
                        nc.gpsimd.dma_start(out=agin[:], in_=snd[:])
                        nc.gpsimd.collective_compute(
                            "AllGather",
                            ALU.bypass,
                            replica_groups=PAIRS,
                            ins=[agin[:].opt()],
                            outs=[agout[:].opt()],
                        )
                    e01 = work.tile([HID, 2, W], BF16, tag="e01", name=f"e01_{t}")
                    if use_coll:
                        nc.gpsimd.dma_start(
                            out=e01[:], in_=agout[:].rearrange("(j p) w -> p j w", p=HID)
                        )
                    else:
                        nc.vector.memset(e01[:], 0.0)

                # ---- interior rows
                ogi = work.tile([HID, NR - 2, W], F32, tag="ogi", name=f"ogi_{t}")
                nc.scalar.activation(
                    ogi[:], p[2][:, ISL, :], AFT.Sigmoid, bias=cb_sb[:, 2:3]
                )
                ui = work.tile([HID, NR - 2, W], F32, tag="ui", name=f"ui_{t}")
                nc.vector.tensor_mul(ui[:], fg[:, ISL, :], cst[:, ISL, :])
                vi = work.tile([HID, NR - 2, W], F32, tag="vi", name=f"vi_{t}")
                nc.vector.tensor_mul(vi[:], ig[:, ISL, :], gg[:, ISL, :])
                nc.vector.tensor_add(cst[:, ISL, :], ui[:], vi[:])
                tchi = work.tile([HID, NR - 2, W], F32, tag="tchi", name=f"tchi_{t}")
                nc.scalar.activation(tchi[:], cst[:, ISL, :], AFT.Tanh)
                nc.vector.tensor_mul(
                    hn[:, 2:NR, 1 : W + 1], ogi[:], tchi[:]
                )
                nc.vector.tensor_reduce(
                    hsum[:, t : t + 1],
                    hn[:, 1 : NR + 1, 1 : W + 1],
                    axis=AXT.XY,
                    op=ALU.add,
                )
                if use_coll and n_steps >= 4 and t == n_steps - 2:
                    # pair-exchange of the pooled sums for all finished steps;
                    # overlaps with the last scan step (which has no halo
                    # exchange of its own, so the CC queue is free)
                    npre = n_steps - 1
                    hg1i = dram.tile([HID, npre], F32, tag="hg1i", name="hg1i")
                    hg1o = dram.tile([2 * HID, npre], F32, tag="hg1o", name="hg1o")
                    nc.gpsimd.dma_start(out=hg1i[:], in_=hsum[:, 0:npre])
                    nc.gpsimd.collective_compute(
                        "AllGather", ALU.bypass, replica_groups=PAIRS,
                        ins=[hg1i[:].opt()], outs=[hg1o[:].opt()],
                    )
                    eh1 = work.tile([HID, 2, npre], F32, tag="eh1", name="eh1")
                    nc.gpsimd.dma_start(
                        out=eh1[:], in_=hg1o[:].rearrange("(j p) w -> p j w", p=HID)
                    )

                if t + 1 < n_steps:
                    # ghost rows from the exchange (masked per core)
                    nc.vector.tensor_scalar_mul(hn[:, 0, 1 : W + 1], e01[:, 0, :], q0)
                    nc.vector.tensor_scalar_mul(
                        hn[:, NR + 1, 1 : W + 1], e01[:, 1, :], q1
                    )

                if t + 2 < n_steps:
                    ps[t + 2] = [
                        psum.tile([HID, NR, W], F32, tag=f"ps{g}", name=f"ps{g}_{t+2}")
                        for g in range(4)
                    ]
                    xpart(ps[t + 2], xt[t + 2])

                hcur = hn

            # ---- head: pair-reduce the pooled sums, then the two FC layers
            fsum = work.tile([HID, S], F32, tag="fsum", name="fsum")
            if use_coll and n_steps >= 4:
                npre = n_steps - 1
                nc.vector.tensor_add(fsum[:, 0:npre], eh1[:, 0, :], eh1[:, 1, :])
                hg2i = dram.tile([HID, 1], F32, tag="hg2i", name="hg2i")
                hg2o = dram.tile([2 * HID, 1], F32, tag="hg2o", name="hg2o")
                nc.gpsimd.dma_start(out=hg2i[:], in_=hsum[:, npre:n_steps])
                nc.gpsimd.collective_compute(
                    "AllGather", ALU.bypass, replica_groups=PAIRS,
                    ins=[hg2i[:].opt()], outs=[hg2o[:].opt()],
                )
                eh2 = work.tile([HID, 2, 1], F32, tag="eh2", name="eh2")
                nc.gpsimd.dma_start(
                    out=eh2[:], in_=hg2o[:].rearrange("(j p) w -> p j w", p=HID)
                )
                nc.vector.tensor_add(
                    fsum[:, npre:n_steps], eh2[:, 0, :], eh2[:, 1, :]
                )
                if n_steps < S:
                    nc.vector.memset(fsum[:, n_steps:S], 0.0)
            else:
                nc.vector.tensor_copy(fsum[:], hsum[:])
            pf = psum.tile([C, S], F32, tag="ps0", name="pf")
            nc.tensor.matmul(pf[:], fcw_sb[:], fsum[:], start=True, stop=True)
            feat = work.tile([C, S], F32, tag="feat", name="feat")
            nc.scalar.activation(feat[:], pf[:], AFT.Relu, bias=fcb_sb[:, 0:1])
            ph = psum.tile([2, S], F32, tag="ps1", name="ph")
            nc.tensor.matmul(ph[:], fhw_sb[:], feat[:], start=True, stop=True)
            oa = work.tile([2, S], F32, tag="oa", name="oa")
            nc.scalar.activation(oa[:], ph[:], AFT.Identity, bias=fhb_sb[:, 0:1])
            nc.sync.dma_start(out=out, in_=oa[:])

    nc.compile()
    return nc


def _prep_in_maps(x, conv_w, conv_b, init_h, init_c, fc_w, fc_b, fco_w, fco_b, fca_w, fca_b):
    f = np.float32
    cw = np.asarray(conv_w, f).reshape(4, HID, C + HID, 3, 3)  # [g, m, kin, dy, dx]
    # lhsT layout [k, g, tap, m]
    wx = np.ascontiguousarray(
        cw[:, :, :C].transpose(2, 0, 3, 4, 1).reshape(C, 4, 9, HID)
    ).astype(NP_BF16)
    wh = np.ascontiguousarray(
        cw[:, :, C:].transpose(2, 0, 3, 4, 1).reshape(HID, 4, 9, HID)
    ).astype(NP_BF16)
    cb = np.ascontiguousarray(np.asarray(conv_b, f).reshape(4, HID).T)  # [HID, 4]
    ih = np.asarray(init_h, f).reshape(HID, 1)
    ic = np.asarray(init_c, f).reshape(HID, 1)
    # fold the 1/(H*W) spatial mean into fc_w;  lhsT = fc_w.T
    fcw = np.ascontiguousarray(np.asarray(fc_w, f).T / f(H * W))  # [HID, C]
    fcb = np.asarray(fc_b, f).reshape(C, 1)
    fhw = np.ascontiguousarray(
        np.stack([np.asarray(fco_w, f)[0], np.asarray(fca_w, f)[0]], axis=1)
    )  # [C, 2]
    fhb = np.array([[np.asarray(fco_b, f)[0]], [np.asarray(fca_b, f)[0]]], f)  # [2, 1]

    x = np.asarray(x, f)
    in_maps = []
    for b in range(B):
        for half in range(2):
            xs = np.zeros((S, C, BR, BC), f)
            if half == 0:  # top: image rows -1..16, row -1 is zero padding
                xs[:, :, 1:BR, 1 : W + 1] = x[b][:, :, 0 : NR + 1, :]
                m = [1.0, 0.0, 0.0, 1.0]
            else:  # bottom: image rows 15..32, row 32 is zero padding
                xs[:, :, 0 : BR - 1, 1 : W + 1] = x[b][:, :, NR - 1 : H, :]
                m = [0.0, 1.0, 1.0, 0.0]
            msk = np.ascontiguousarray(np.broadcast_to(np.array(m, f), (128, 4)))
            in_maps.append(
                dict(
                    xs=xs.astype(NP_BF16), wx=wx, wh=wh, cb=cb, ih=ih, ic=ic,
                    fcw=fcw, fcb=fcb, fhw=fhw, fhb=fhb, msk=msk,
                )
            )
    return in_maps


def _numpy_ref(x, conv_w, conv_b, init_h, init_c, fc_w, fc_b, fco_w, fco_b, fca_w, fca_b):
    f = np.float32
    x = np.asarray(x, f)
    b_, s_, c_, h_, w_ = x.shape
    hid = init_h.shape[0]
    hcur = np.broadcast_to(np.asarray(init_h, f)[None, :, None, None], (b_, hid, h_, w_)).copy()
    cst = np.broadcast_to(np.asarray(init_c, f)[None, :, None, None], (b_, hid, h_, w_)).copy()
    wxy = np.asarray(conv_w, f)  # [4h, c+hid, 3, 3]
    feats = np.zeros((b_, s_, hid), f)

    def conv(z):
        zp = np.pad(z, ((0, 0), (0, 0), (1, 1), (1, 1)))
        out = np.zeros((b_, 4 * hid, h_, w_), f)
        for dy in range(3):
            for dx in range(3):
                out += np.einsum(
                    "ok,bkhw->bohw", wxy[:, :, dy, dx],
                    zp[:, :, dy : dy + h_, dx : dx + w_],
                    optimize=True,
                )
        return out + np.asarray(conv_b, f)[None, :, None, None]

    def sig(v):
        return 1.0 / (1.0 + np.exp(-v))

    for t in range(s_):
        z = np.concatenate([x[:, t], hcur], axis=1)
        g = conv(z)
        i, fo, o, gg = np.split(g, 4, axis=1)
        cst = sig(fo) * cst + sig(i) * np.tanh(gg)
        hcur = sig(o) * np.tanh(cst)
        feats[:, t] = hcur.mean(axis=(2, 3))
    feat = np.maximum(feats @ np.asarray(fc_w, f).T + np.asarray(fc_b, f), 0.0)
    offset = feat @ np.asarray(fco_w, f).T + np.asarray(fco_b, f)
    angle = feat @ np.asarray(fca_w, f).T + np.asarray(fca_b, f)
    return offset.astype(f), angle.astype(f)


def kernel(x, conv_w, conv_b, init_h, init_c, fc_w, fc_b, fco_w, fco_b, fca_w, fca_b,
           _return_bass_results=False, _trace=False, _use_coll=True):
    args = (x, conv_w, conv_b, init_h, init_c, fc_w, fc_b, fco_w, fco_b, fca_w, fca_b)
    try:
        key = ("nc", _use_coll)
        if key not in _cache:
            _cache[key] = _build(_use_coll)
        nc = _cache[key]
        in_maps = _prep_in_maps(*args)
        res = run_bass_kernel_spmd(nc, in_maps, list(range(8)), trace=_trace)
        offset = np.zeros((B, S, 1), np.float32)
        angle = np.zeros((B, S, 1), np.float32)
        for b in range(B):
            o = res.results[2 * b]["out"]
            offset[b, :, 0] = o[0]
            angle[b, :, 0] = o[1]
    except Exception:
        if _return_bass_results:
            raise
        o, a = _numpy_ref(*args)
        return o, a
    if _return_bass_results:
        return (offset, angle), res
    return (offset, angle)


# revision 12
# speedup vs baseline: 1.0067x; 1.0067x over previous
"""ConvLSTM + FC head on 8 Trainium2 NeuronCores.

Reference computation (see problem): x [B=4, S=32, C=128, H=32, W=32],
ConvLSTM with HID=128, 3x3 SAME conv over concat(x_t, h), scanned over S;
then spatial mean -> relu(fc) -> two scalar heads -> (offset, angle),
each [B, S, 1].

Sharding: 8 cores = 4 batch elements x 2-way split of the H dimension
(rows 0..15 / 16..31).  Each step a core computes its 16 rows of the new
hidden state; the single-row halo of h needed by the 3x3 conv is exchanged
between the pair through a 2-rank AllGather.

Schedule (per step, all conv matmuls bf16 into fp32 PSUM):
  tensor:  hpart-interior(t) | hpart-boundary(t) | xpart(t+2)
  scalar:  4 gate activations, tanh(c) for boundary rows then interior
  vector:  boundary-row state update first -> snd row -> AllGather launch
           (hidden behind xpart(t+2) + hpart-interior(t+1)), then interior
           state update, pooled-sum reduce, ghost-row writes from the
           AllGather result.
The boundary rows {0,15} of the gates are accumulated last (stride-15
matmuls) so the interior work never waits on the halo exchange, and the
exchange result is only needed by the *boundary* matmuls of step t+1,
one full tensor block later.
"""

import numpy as np
import ml_dtypes

import concourse.bass as bass
from concourse import bacc
import concourse.mybir as mybir
import concourse.tile as tile
from concourse.bass_utils import run_bass_kernel_spmd

B, S, C, H, W = 4, 32, 128, 32, 32
HID = 128
NR = 16                  # own rows per core
BR, BC = NR + 2, W + 2   # buffered rows/cols (halo rows + zero-pad cols)
PAIRS = [[0, 1], [2, 3], [4, 5], [6, 7]]
F32 = mybir.dt.float32
BF16 = mybir.dt.bfloat16
NP_BF16 = ml_dtypes.bfloat16
AFT = mybir.ActivationFunctionType
ALU = mybir.AluOpType
AXT = mybir.AxisListType

_cache = {}

# boundary rows of the 16-row slab: {0, 15} via stride-15 slices
BSL = slice(0, NR, NR - 1)          # ps/cst rows {0,15}
ISL = slice(1, NR - 1)              # ps/cst rows 1..14


def _build(use_coll=True, n_steps=S):
    nc = bacc.Bacc("TRN2", target_bir_lowering=False, debug=False, num_devices=8)
    xs = nc.dram_tensor("xs", [S, C, BR, BC], BF16, kind="ExternalInput").ap()
    wx = nc.dram_tensor("wx", [C, 4, 9, HID], BF16, kind="ExternalInput").ap()
    wh = nc.dram_tensor("wh", [HID, 4, 9, HID], BF16, kind="ExternalInput").ap()
    cb = nc.dram_tensor("cb", [HID, 4], F32, kind="ExternalInput").ap()
    ih = nc.dram_tensor("ih", [HID, 1], F32, kind="ExternalInput").ap()
    ic = nc.dram_tensor("ic", [HID, 1], F32, kind="ExternalInput").ap()
    fcw = nc.dram_tensor("fcw", [HID, C], F32, kind="ExternalInput").ap()
    fcb = nc.dram_tensor("fcb", [C, 1], F32, kind="ExternalInput").ap()
    fhw = nc.dram_tensor("fhw", [C, 2], F32, kind="ExternalInput").ap()
    fhb = nc.dram_tensor("fhb", [2, 1], F32, kind="ExternalInput").ap()
    msk = nc.dram_tensor("msk", [128, 4], F32, kind="ExternalInput").ap()
    out = nc.dram_tensor("out", [2, S], F32, kind="ExternalOutput").ap()

    with tile.TileContext(nc) as tc:
        with (
            tc.tile_pool(name="consts", bufs=1) as consts,
            tc.tile_pool(name="xpool", bufs=3) as xpool,
            tc.tile_pool(name="work", bufs=2) as work,
            tc.tile_pool(name="state", bufs=1) as state,
            tc.tile_pool(name="psum", bufs=2, space="PSUM") as psum,
            tc.tile_pool(name="dram", bufs=2, space="DRAM") as dram,
        ):
            # ---- small constants + first x tiles first (unblock init + MMs)
            cb_sb = consts.tile([HID, 4], F32, name="cb_sb")
            nc.sync.dma_start(out=cb_sb[:], in_=cb)
            ih_sb = consts.tile([HID, 1], F32, name="ih_sb")
            nc.sync.dma_start(out=ih_sb[:], in_=ih)
            ic_sb = consts.tile([HID, 1], F32, name="ic_sb")
            nc.sync.dma_start(out=ic_sb[:], in_=ic)
            msk_sb = consts.tile([128, 4], F32, name="msk_sb")
            nc.sync.dma_start(out=msk_sb[:], in_=msk)
            xt = {}
            for t0 in range(2):
                xt[t0] = xpool.tile([C, BR, BC], BF16, tag="x", name=f"x_{t0}")
                nc.sync.dma_start(out=xt[t0][:], in_=xs[t0])
            wx_sb = consts.tile([C, 4, 9, HID], BF16, name="wx_sb")
            nc.sync.dma_start(out=wx_sb[:], in_=wx)
            wh_sb = consts.tile([HID, 4, 9, HID], BF16, name="wh_sb")
            nc.sync.dma_start(out=wh_sb[:], in_=wh)
            fcw_sb = consts.tile([HID, C], F32, name="fcw_sb")
            nc.sync.dma_start(out=fcw_sb[:], in_=fcw)
            fcb_sb = consts.tile([C, 1], F32, name="fcb_sb")
            nc.sync.dma_start(out=fcb_sb[:], in_=fcb)
            fhw_sb = consts.tile([C, 2], F32, name="fhw_sb")
            nc.sync.dma_start(out=fhw_sb[:], in_=fhw)
            fhb_sb = consts.tile([2, 1], F32, name="fhb_sb")
            nc.sync.dma_start(out=fhb_sb[:], in_=fhb)

            # ---- collective-path warmup: the CC core's first-use queue setup
            # (~8us) is paid once PER payload configuration, so issue a dummy
            # AllGather for each config used later, all off the critical path
            if use_coll:
                wrmb = work.tile([HID, W], BF16, tag="wrmb", name="wrmb")
                nc.vector.memset(wrmb[:], 0.0)
                wrmf = work.tile([HID, S], F32, tag="wrmf", name="wrmf")
                nc.vector.memset(wrmf[:], 0.0)
                wag_i = dram.tile([HID, W], BF16, tag="agin", name="wag_i")
                wag_o = dram.tile([2 * HID, W], BF16, tag="agout", name="wag_o")
                nc.gpsimd.dma_start(out=wag_i[:], in_=wrmb[:])
                nc.gpsimd.collective_compute(
                    "AllGather", ALU.bypass, replica_groups=PAIRS,
                    ins=[wag_i[:].opt()], outs=[wag_o[:].opt()],
                )
                if n_steps >= 4:
                    npre = n_steps - 1
                    whg1_i = dram.tile([HID, npre], F32, tag="hg1i", name="whg1_i")
                    whg1_o = dram.tile([2 * HID, npre], F32, tag="hg1o", name="whg1_o")
                    nc.gpsimd.dma_start(out=whg1_i[:], in_=wrmf[:, 0:npre])
                    nc.gpsimd.collective_compute(
                        "AllGather", ALU.bypass, replica_groups=PAIRS,
                        ins=[whg1_i[:].opt()], outs=[whg1_o[:].opt()],
                    )
                    whg2_i = dram.tile([HID, 1], F32, tag="hg2i", name="whg2_i")
                    whg2_o = dram.tile([2 * HID, 1], F32, tag="hg2o", name="whg2_o")
                    nc.gpsimd.dma_start(out=whg2_i[:], in_=wrmf[:, 0:1])
                    nc.gpsimd.collective_compute(
                        "AllGather", ALU.bypass, replica_groups=PAIRS,
                        ins=[whg2_i[:].opt()], outs=[whg2_o[:].opt()],
                    )

            s0 = msk_sb[:, 0:1]
            s1 = msk_sb[:, 1:2]
            q0 = msk_sb[:, 2:3]
            q1 = msk_sb[:, 3:4]

            ihq0 = consts.tile([HID, 1], F32, name="ihq0")
            nc.vector.tensor_mul(ihq0[:], ih_sb[:], q0)
            ihq1 = consts.tile([HID, 1], F32, name="ihq1")
            nc.vector.tensor_mul(ihq1[:], ih_sb[:], q1)

            hsum = state.tile([HID, S], F32, name="hsum")

            # ---- persistent h buffers (even/odd steps); pad cols zeroed once
            hb = [
                state.tile([HID, BR, BC], BF16, name="h_even"),
                state.tile([HID, BR, BC], BF16, name="h_odd"),
            ]
            cst = state.tile([HID, NR, W], F32, name="cst")
            nc.vector.memset(cst[:], 0.0)
            # h_even holds h(0): interior = ih, ghost rows masked, pads zero
            nc.vector.memset(hb[0][:], 0.0)
            nc.vector.tensor_scalar_add(
                hb[0][:, 1 : NR + 1, 1 : W + 1], cst[:], ih_sb[:, 0:1]
            )
            nc.vector.tensor_scalar_add(
                hb[0][:, 0, 1 : W + 1], cst[:, 0, :], ihq0[:, 0:1]
            )
            nc.vector.tensor_scalar_add(
                hb[0][:, NR + 1, 1 : W + 1], cst[:, 0, :], ihq1[:, 0:1]
            )
            # h_odd: only the pad cols need to start (and stay) zero
            nc.vector.memset(hb[1][:, :, 0:1], 0.0)
            nc.vector.memset(hb[1][:, :, W + 1 : W + 2], 0.0)
            nc.vector.tensor_scalar_add(cst[:], cst[:], ic_sb[:, 0:1])

            def xpart(ps, x):
                for g in range(4):
                    for tap in range(9):
                        dy, dx = divmod(tap, 3)
                        nc.tensor.matmul(
                            ps[g][:],
                            wx_sb[:, g, tap, :],
                            x[:, dy : dy + NR, dx : dx + W],
                            start=(tap == 0),
                            stop=False,
                        )

            def hpart_int(ps, h):
                # output rows 1..14 <- input buffer rows (1+dy)..(14+dy)
                for g in range(4):
                    for tap in range(9):
                        dy, dx = divmod(tap, 3)
                        nc.tensor.matmul(
                            ps[g][:, ISL, :],
                            wh_sb[:, g, tap, :],
                            h[:, 1 + dy : NR - 1 + dy, dx : dx + W],
                            start=False,
                            stop=False,
                        )

            def hpart_bnd(ps, h):
                # output rows {0,15} <- input buffer rows {dy, 15+dy}
                for g in range(4):
                    for tap in range(9):
                        dy, dx = divmod(tap, 3)
                        nc.tensor.matmul(
                            ps[g][:, BSL, :],
                            wh_sb[:, g, tap, :],
                            h[:, dy : dy + NR : NR - 1, dx : dx + W],
                            start=False,
                            stop=(tap == 8),
                        )

            # ---- prologue: conv x-parts of the first two steps
            ps = {}
            for t0 in range(2):
                ps[t0] = [
                    psum.tile([HID, NR, W], F32, tag=f"ps{g}", name=f"ps{g}_{t0}")
                    for g in range(4)
                ]
                xpart(ps[t0], xt[t0])

            hcur = hb[0]
            for t in range(n_steps):
                if t + 2 < n_steps:
                    xt[t + 2] = xpool.tile([C, BR, BC], BF16, tag="x", name=f"x_{t+2}")
                    nc.sync.dma_start(out=xt[t + 2][:], in_=xs[t + 2])
                hn = hb[(t + 1) % 2]
                p = ps.pop(t)
                hpart_int(p, hcur)
                hpart_bnd(p, hcur)

                # ---- gate activations, ordered so the send-row chain can
                # launch ASAP: fg/ig/gg full, og boundary-only early, og
                # interior later
                fg = work.tile([HID, NR, W], F32, tag="fg", name=f"fg_{t}")
                nc.scalar.activation(fg[:], p[1][:], AFT.Sigmoid, bias=cb_sb[:, 1:2])
                ig = work.tile([HID, NR, W], F32, tag="ig", name=f"ig_{t}")
                nc.scalar.activation(ig[:], p[0][:], AFT.Sigmoid, bias=cb_sb[:, 0:1])
                gg = work.tile([HID, NR, W], F32, tag="gg", name=f"gg_{t}")
                nc.scalar.activation(gg[:], p[3][:], AFT.Tanh, bias=cb_sb[:, 3:4])
                ogb = work.tile([HID, 2, W], F32, tag="ogb", name=f"ogb_{t}")
                nc.scalar.activation(
                    ogb[:], p[2][:, BSL, :], AFT.Sigmoid, bias=cb_sb[:, 2:3]
                )

                # ---- boundary rows first: state update -> send row
                ub = work.tile([HID, 2, W], F32, tag="ub", name=f"ub_{t}")
                nc.vector.tensor_mul(ub[:], fg[:, BSL, :], cst[:, BSL, :])
                vb = work.tile([HID, 2, W], F32, tag="vb", name=f"vb_{t}")
                nc.vector.tensor_mul(vb[:], ig[:, BSL, :], gg[:, BSL, :])
                nc.vector.tensor_add(cst[:, BSL, :], ub[:], vb[:])
                tchb = work.tile([HID, 2, W], F32, tag="tchb", name=f"tchb_{t}")
                nc.scalar.activation(tchb[:], cst[:, BSL, :], AFT.Tanh)
                nc.vector.tensor_mul(
                    hn[:, 1 : NR + 1 : NR - 1, 1 : W + 1], ogb[:], tchb[:]
                )

                if t + 1 < n_steps:
                    # send row: top sends its row 16 (image row 15), bottom row 1
                    tmp = work.tile([HID, W], BF16, tag="tmp", name=f"tmp_{t}")
                    nc.vector.tensor_scalar_mul(tmp[:], hn[:, NR, 1 : W + 1], s0)
                    snd = work.tile([HID, W], BF16, tag="snd", name=f"snd_{t}")
                    nc.vector.scalar_tensor_tensor(
                        snd[:], hn[:, 1, 1 : W + 1], s1, tmp[:],
                        op0=ALU.mult, op1=ALU.add,
                    )
                    agin = dram.tile([HID, W], BF16, tag="agin", name=f"agin_{t}")
                    agout = dram.tile([2 * HID, W], BF16, tag="agout", name=f"agout_{t}")
                    if use_coll:
                        nc.gpsimd.dma_start(out=agin[:], in_=snd[:])
                        nc.gpsimd.collective_compute(
                            "AllGather",
                            ALU.bypass,
                            replica_groups=PAIRS,
                            ins=[agin[:].opt()],
                            outs=[agout[:].opt()],
                        )
                    e01 = work.tile([HID, 2, W], BF16, tag="e01", name=f"e01_{t}")
                    if use_coll:
                        nc.gpsimd.dma_start(
                            out=e01[:], in_=agout[:].rearrange("(j p) w -> p j w", p=HID)
                        )
                    else:
                        nc.vector.memset(e01[:], 0.0)

                # ---- interior rows
                ogi = work.tile([HID, NR - 2, W], F32, tag="ogi", name=f"ogi_{t}")
                nc.scalar.activation(
                    ogi[:], p[2][:, ISL, :], AFT.Sigmoid, bias=cb_sb[:, 2:3]
                )
                ui = work.tile([HID, NR - 2, W], F32, tag="ui", name=f"ui_{t}")
                nc.vector.tensor_mul(ui[:], fg[:, ISL, :], cst[:, ISL, :])
                vi = work.tile([HID, NR - 2, W], F32, tag="vi", name=f"vi_{t}")
                nc.vector.tensor_mul(vi[:], ig[:, ISL, :], gg[:, ISL, :])
                nc.vector.tensor_add(cst[:, ISL, :], ui[:], vi[:])
                tchi = work.tile([HID, NR - 2, W], F32, tag="tchi", name=f"tchi_{t}")
                nc.scalar.activation(tchi[:], cst[:, ISL, :], AFT.Tanh)
                nc.vector.tensor_mul(
                    hn[:, 2:NR, 1 : W + 1], ogi[:], tchi[:]
                )
                nc.vector.tensor_reduce(
                    hsum[:, t : t + 1],
                    hn[:, 1 : NR + 1, 1 : W + 1],
                    axis=AXT.XY,
                    op=ALU.add,
                )
                if use_coll and n_steps >= 4 and t == n_steps - 2:
                    # pair-exchange of the pooled sums for all finished steps;
                    # overlaps with the last scan step (which has no halo
                    # exchange of its own, so the CC queue is free)
                    npre = n_steps - 1
                    hg1i = dram.tile([HID, npre], F32, tag="hg1i", name="hg1i")
                    hg1o = dram.tile([2 * HID, npre], F32, tag="hg1o", name="hg1o")
                    nc.gpsimd.dma_start(out=hg1i[:], in_=hsum[:, 0:npre])
                    nc.gpsimd.collective_compute(
                        "AllGather", ALU.bypass, replica_groups=PAIRS,
                        ins=[hg1i[:].opt()], outs=[hg1o[:].opt()],
                    )
                    eh1 = work.tile([HID, 2, npre], F32, tag="eh1", name="eh1")
                    nc.gpsimd.dma_start(
                        out=eh1[:], in_=hg1o[:].rearrange("(j p) w -> p j w", p=HID)
                    )

                if t + 1 < n_steps:
                    # ghost rows from the exchange (masked per core)
                    nc.vector.tensor_scalar_mul(hn[:, 0, 1 : W + 1], e01[:, 0, :], q0)
                    nc.vector.tensor_scalar_mul(
                        hn[:, NR + 1, 1 : W + 1], e01[:, 1, :], q1
                    )

                if t + 2 < n_steps:
                    ps[t + 2] = [
                        psum.tile([HID, NR, W], F32, tag=f"ps{g}", name=f"ps{g}_{t+2}")
                        for g in range(4)
                    ]
                    xpart(ps[t + 2], xt[t + 2])

                hcur = hn

            # ---- head: pair-reduce the pooled sums, then the two FC layers
            fsum = work.tile([HID, S], F32, tag="fsum", name="fsum")
            if use_coll and n_steps >= 4:
                npre = n_steps - 1
                nc.vector.tensor_add(fsum[:, 0:npre], eh1[:, 0, :], eh1[:, 1, :])
                hg2i = dram.tile([HID, 1], F32, tag="hg2i", name="hg2i")
                hg2o = dram.tile([2 * HID, 1], F32, tag="hg2o", name="hg2o")
                nc.gpsimd.dma_start(out=hg2i[:], in_=hsum[:, npre:n_steps])
                nc.gpsimd.collective_compute(
                    "AllGather", ALU.bypass, replica_groups=PAIRS,
                    ins=[hg2i[:].opt()], outs=[hg2o[:].opt()],
                )
                eh2 = work.tile([HID, 2, 1], F32, tag="eh2", name="eh2")
                nc.gpsimd.dma_start(
                    out=eh2[:], in_=hg2o[:].rearrange("(j p) w -> p j w", p=HID)
                )
                nc.vector.tensor_add(
                    fsum[:, npre:n_steps], eh2[:, 0, :], eh2[:, 1, :]
                )
                if n_steps < S:
                    nc.vector.memset(fsum[:, n_steps:S], 0.0)
            else:
                nc.vector.tensor_copy(fsum[:], hsum[:])
            pf = psum.tile([C, S], F32, tag="ps0", name="pf")
            nc.tensor.matmul(pf[:], fcw_sb[:], fsum[:], start=True, stop=True)
            feat = work.tile([C, S], F32, tag="feat", name="feat")
            nc.scalar.activation(feat[:], pf[:], AFT.Relu, bias=fcb_sb[:, 0:1])
            ph = psum.tile([2, S], F32, tag="ps1", name="ph")
            nc.tensor.matmul(ph[:], fhw_sb[:], feat[:], start=True, stop=True)
            oa = work.tile([2, S], F32, tag="oa", name="oa")
            nc.scalar.activation(oa[:], ph[:], AFT.Identity, bias=fhb_sb[:, 0:1])
            nc.sync.dma_start(out=out, in_=oa[:])

    nc.compile()
    return nc


def _prep_in_maps(x, conv_w, conv_b, init_h, init_c, fc_w, fc_b, fco_w, fco_b, fca_w, fca_b):
    f = np.float32
    cw = np.asarray(conv_w, f).reshape(4, HID, C + HID, 3, 3)  # [g, m, kin, dy, dx]
    # lhsT layout [k, g, tap, m]
    wx = np.ascontiguousarray(
        cw[:, :, :C].transpose(2, 0, 3, 4, 1).reshape(C, 4, 9, HID)
    ).astype(NP_BF16)
    wh = np.ascontiguousarray(
        cw[:, :, C:].transpose(2, 0, 3, 4, 1).reshape(HID, 4, 9, HID)
    ).astype(NP_BF16)
    cb = np.ascontiguousarray(np.asarray(conv_b, f).reshape(4, HID).T)  # [HID, 4]
    ih = np.asarray(init_h, f).reshape(HID, 1)
    ic = np.asarray(init_c, f).reshape(HID, 1)
    # fold the 1/(H*W) spatial mean into fc_w;  lhsT = fc_w.T
    fcw = np.ascontiguousarray(np.asarray(fc_w, f).T / f(H * W))  # [HID, C]
    fcb = np.asarray(fc_b, f).reshape(C, 1)
    fhw = np.ascontiguousarray(
        np.stack([np.asarray(fco_w, f)[0], np.asarray(fca_w, f)[0]], axis=1)
    )  # [C, 2]
    fhb = np.array([[np.asarray(fco_b, f)[0]], [np.asarray(fca_b, f)[0]]], f)  # [2, 1]

    x = np.asarray(x, f)
    in_maps = []
    for b in range(B):
        for half in range(2):
            xs = np.zeros((S, C, BR, BC), f)
            if half == 0:  # top: image rows -1..16, row -1 is zero padding
                xs[:, :, 1:BR, 1 : W + 1] = x[b][:, :, 0 : NR + 1, :]
                m = [1.0, 0.0, 0.0, 1.0]
            else:  # bottom: image rows 15..32, row 32 is zero padding
                xs[:, :, 0 : BR - 1, 1 : W + 1] = x[b][:, :, NR - 1 : H, :]
                m = [0.0, 1.0, 1.0, 0.0]
            msk = np.ascontiguousarray(np.broadcast_to(np.array(m, f), (128, 4)))
            in_maps.append(
                dict(
                    xs=xs.astype(NP_BF16), wx=wx, wh=wh, cb=cb, ih=ih, ic=ic,
                    fcw=fcw, fcb=fcb, fhw=fhw, fhb=fhb, msk=msk,
                )
            )
    return in_maps


def _numpy_ref(x, conv_w, conv_b, init_h, init_c, fc_w, fc_b, fco_w, fco_b, fca_w, fca_b):
    f = np.float32
    x = np.asarray(x, f)
    b_, s_, c_, h_, w_ = x.shape
    hid = init_h.shape[0]
    hcur = np.broadcast_to(np.asarray(init_h, f)[None, :, None, None], (b_, hid, h_, w_)).copy()
    cst = np.broadcast_to(np.asarray(init_c, f)[None, :, None, None], (b_, hid, h_, w_)).copy()
    wxy = np.asarray(conv_w, f)  # [4h, c+hid, 3, 3]
    feats = np.zeros((b_, s_, hid), f)

    def conv(z):
        zp = np.pad(z, ((0, 0), (0, 0), (1, 1), (1, 1)))
        out = np.zeros((b_, 4 * hid, h_, w_), f)
        for dy in range(3):
            for dx in range(3):
                out += np.einsum(
                    "ok,bkhw->bohw", wxy[:, :, dy, dx],
                    zp[:, :, dy : dy + h_, dx : dx + w_],
                    optimize=True,
                )
        return out + np.asarray(conv_b, f)[None, :, None, None]

    def sig(v):
        return 1.0 / (1.0 + np.exp(-v))

    for t in range(s_):
        z = np.concatenate([x[:, t], hcur], axis=1)
        g = conv(z)
        i, fo, o, gg = np.split(g, 4, axis=1)
        cst = sig(fo) * cst + sig(i) * np.tanh(gg)
        hcur = sig(o) * np.tanh(cst)
        feats[:, t] = hcur.mean(axis=(2, 3))
    feat = np.maximum(feats @ np.asarray(fc_w, f).T + np.asarray(fc_b, f), 0.0)
    offset = feat @ np.asarray(fco_w, f).T + np.asarray(fco_b, f)
    angle = feat @ np.asarray(fca_w, f).T + np.asarray(fca_b, f)
    return offset.astype(f), angle.astype(f)


def kernel(x, conv_w, conv_b, init_h, init_c, fc_w, fc_b, fco_w, fco_b, fca_w, fca_b,
           _return_bass_results=False, _trace=False, _use_coll=True):
    args = (x, conv_w, conv_b, init_h, init_c, fc_w, fc_b, fco_w, fco_b, fca_w, fca_b)
    try:
        key = ("nc", _use_coll)
        if key not in _cache:
            _cache[key] = _build(_use_coll)
        nc = _cache[key]
        in_maps = _prep_in_maps(*args)
        res = run_bass_kernel_spmd(nc, in_maps, list(range(8)), trace=_trace)
        offset = np.zeros((B, S, 1), np.float32)
        angle = np.zeros((B, S, 1), np.float32)
        for b in range(B):
            o = res.results[2 * b]["out"]
            offset[b, :, 0] = o[0]
            angle[b, :, 0] = o[1]
    except Exception:
        if _return_bass_results:
            raise
        o, a = _numpy_ref(*args)
        return o, a
    if _return_bass_results:
        return (offset, angle), res
    return (offset, angle)


# revision 16
# speedup vs baseline: 1.0477x; 1.0408x over previous
"""ConvLSTM + FC head on 8 Trainium2 NeuronCores.

Reference computation (see problem): x [B=4, S=32, C=128, H=32, W=32],
ConvLSTM with HID=128, 3x3 SAME conv over concat(x_t, h), scanned over S;
then spatial mean -> relu(fc) -> two scalar heads -> (offset, angle),
each [B, S, 1].

Sharding: 8 cores = 4 batch elements x 2-way split of the H dimension
(rows 0..15 / 16..31).  Each step a core computes its 16 rows of the new
hidden state; the single-row halo of h needed by the 3x3 conv is exchanged
between the pair through a 2-rank AllGather.

Schedule (per step, all conv matmuls bf16 into fp32 PSUM):
  tensor:  hpart-interior(t) | hpart-boundary(t) | xpart(t+2)
  scalar:  4 gate activations, tanh(c) for boundary rows then interior
  vector:  boundary-row state update first -> snd row -> AllGather launch
           (hidden behind xpart(t+2) + hpart-interior(t+1)), then interior
           state update, pooled-sum reduce, ghost-row writes from the
           AllGather result.
The boundary rows {0,15} of the gates are accumulated last (stride-15
matmuls) so the interior work never waits on the halo exchange, and the
exchange result is only needed by the *boundary* matmuls of step t+1,
one full tensor block later.
"""

import numpy as np
import ml_dtypes

import concourse.bass as bass
from concourse import bacc
import concourse.mybir as mybir
import concourse.tile as tile
from concourse.bass_utils import run_bass_kernel_spmd

B, S, C, H, W = 4, 32, 128, 32, 32
HID = 128
NR = 16                  # own rows per core
BR, BC = NR + 2, W + 2   # buffered rows/cols (halo rows + zero-pad cols)
PAIRS = [[0, 1], [2, 3], [4, 5], [6, 7]]
F32 = mybir.dt.float32
BF16 = mybir.dt.bfloat16
NP_BF16 = ml_dtypes.bfloat16
AFT = mybir.ActivationFunctionType
ALU = mybir.AluOpType
AXT = mybir.AxisListType

_cache = {}

# boundary rows of the 16-row slab: {0, 15} via stride-15 slices
BSL = slice(0, NR, NR - 1)          # ps/cst rows {0,15}
ISL = slice(1, NR - 1)              # ps/cst rows 1..14


def _build(use_coll=True, n_steps=S):
    nc = bacc.Bacc("TRN2", target_bir_lowering=False, debug=False, num_devices=8)
    xs = nc.dram_tensor("xs", [S, C, BR, BC], BF16, kind="ExternalInput").ap()
    wx = nc.dram_tensor("wx", [C, 4, 9, HID], BF16, kind="ExternalInput").ap()
    wh = nc.dram_tensor("wh", [HID, 4, 9, HID], BF16, kind="ExternalInput").ap()
    cb = nc.dram_tensor("cb", [HID, 4], F32, kind="ExternalInput").ap()
    ih = nc.dram_tensor("ih", [HID, 1], F32, kind="ExternalInput").ap()
    ic = nc.dram_tensor("ic", [HID, 1], F32, kind="ExternalInput").ap()
    fcw = nc.dram_tensor("fcw", [HID, C], F32, kind="ExternalInput").ap()
    fcb = nc.dram_tensor("fcb", [C, 1], F32, kind="ExternalInput").ap()
    fhw = nc.dram_tensor("fhw", [C, 2], F32, kind="ExternalInput").ap()
    fhb = nc.dram_tensor("fhb", [2, 1], F32, kind="ExternalInput").ap()
    msk = nc.dram_tensor("msk", [128, 4], F32, kind="ExternalInput").ap()
    out = nc.dram_tensor("out", [2, S], F32, kind="ExternalOutput").ap()

    with tile.TileContext(nc) as tc:
        with (
            tc.tile_pool(name="consts", bufs=1) as consts,
            tc.tile_pool(name="xpool", bufs=3) as xpool,
            tc.tile_pool(name="work", bufs=2) as work,
            tc.tile_pool(name="state", bufs=1) as state,
            tc.tile_pool(name="psum", bufs=2, space="PSUM") as psum,
            tc.tile_pool(name="dram", bufs=2, space="DRAM") as dram,
        ):
            # ---- small constants + first x tiles first (unblock init + MMs)
            cb_sb = consts.tile([HID, 4], F32, name="cb_sb")
            nc.sync.dma_start(out=cb_sb[:], in_=cb)
            ih_sb = consts.tile([HID, 1], F32, name="ih_sb")
            nc.sync.dma_start(out=ih_sb[:], in_=ih)
            ic_sb = consts.tile([HID, 1], F32, name="ic_sb")
            nc.sync.dma_start(out=ic_sb[:], in_=ic)
            msk_sb = consts.tile([128, 4], F32, name="msk_sb")
            nc.sync.dma_start(out=msk_sb[:], in_=msk)
            xt = {}
            for t0 in range(2):
                xt[t0] = xpool.tile([C, BR, BC], BF16, tag="x", name=f"x_{t0}")
                nc.sync.dma_start(out=xt[t0][:], in_=xs[t0])
            # weights split per-gate across 4 DMA queues (~100GB/s each)
            wx_sb = consts.tile([C, 4, 9, HID], BF16, name="wx_sb")
            wh_sb = consts.tile([HID, 4, 9, HID], BF16, name="wh_sb")
            qsx = [nc.sync, nc.scalar, nc.gpsimd, nc.sync]
            qsh = [nc.scalar, nc.gpsimd, nc.sync, nc.scalar]
            for g in range(4):
                qsx[g].dma_start(out=wx_sb[:, g], in_=wx[:, g])
            for g in range(4):
                qsh[g].dma_start(out=wh_sb[:, g], in_=wh[:, g])
            fcw_sb = consts.tile([HID, C], F32, name="fcw_sb")
            nc.sync.dma_start(out=fcw_sb[:], in_=fcw)
            fcb_sb = consts.tile([C, 1], F32, name="fcb_sb")
            nc.sync.dma_start(out=fcb_sb[:], in_=fcb)
            fhw_sb = consts.tile([C, 2], F32, name="fhw_sb")
            nc.sync.dma_start(out=fhw_sb[:], in_=fhw)
            fhb_sb = consts.tile([2, 1], F32, name="fhb_sb")
            nc.sync.dma_start(out=fhb_sb[:], in_=fhb)

            # ---- collective-path warmup: the CC core's first-use queue setup
            # (~8us) is paid once PER payload configuration, so issue a dummy
            # AllGather for each config used later, all off the critical path
            if use_coll:
                wrmb = work.tile([HID, W], BF16, tag="wrmb", name="wrmb")
                nc.vector.memset(wrmb[:], 0.0)
                wrmf = work.tile([HID, S], F32, tag="wrmf", name="wrmf")
                nc.vector.memset(wrmf[:], 0.0)
                wag_i = dram.tile([HID, W], BF16, tag="agin", name="wag_i")
                wag_o = dram.tile([2 * HID, W], BF16, tag="agout", name="wag_o")
                nc.gpsimd.dma_start(out=wag_i[:], in_=wrmb[:])
                nc.gpsimd.collective_compute(
                    "AllGather", ALU.bypass, replica_groups=PAIRS,
                    ins=[wag_i[:].opt()], outs=[wag_o[:].opt()],
                )
                if n_steps >= 4:
                    npre = n_steps - 1
                    whg1_i = dram.tile([HID, npre], F32, tag="hg1i", name="whg1_i")
                    whg1_o = dram.tile([2 * HID, npre], F32, tag="hg1o", name="whg1_o")
                    nc.gpsimd.dma_start(out=whg1_i[:], in_=wrmf[:, 0:npre])
                    nc.gpsimd.collective_compute(
                        "AllGather", ALU.bypass, replica_groups=PAIRS,
                        ins=[whg1_i[:].opt()], outs=[whg1_o[:].opt()],
                    )


            s0 = msk_sb[:, 0:1]
            s1 = msk_sb[:, 1:2]
            q0 = msk_sb[:, 2:3]
            q1 = msk_sb[:, 3:4]

            ihq0 = consts.tile([HID, 1], F32, name="ihq0")
            nc.vector.tensor_mul(ihq0[:], ih_sb[:], q0)
            ihq1 = consts.tile([HID, 1], F32, name="ihq1")
            nc.vector.tensor_mul(ihq1[:], ih_sb[:], q1)

            hsum = state.tile([HID, S], F32, name="hsum")

            # ---- persistent h buffers (even/odd steps); pad cols zeroed once
            hb = [
                state.tile([HID, BR, BC], BF16, name="h_even"),
                state.tile([HID, BR, BC], BF16, name="h_odd"),
            ]
            cst = state.tile([HID, NR, W], F32, name="cst")
            nc.vector.memset(cst[:], 0.0)
            # h_even holds h(0): interior = ih, ghost rows masked, pads zero
            nc.vector.memset(hb[0][:], 0.0)
            nc.vector.tensor_scalar_add(
                hb[0][:, 1 : NR + 1, 1 : W + 1], cst[:], ih_sb[:, 0:1]
            )
            nc.vector.tensor_scalar_add(
                hb[0][:, 0, 1 : W + 1], cst[:, 0, :], ihq0[:, 0:1]
            )
            nc.vector.tensor_scalar_add(
                hb[0][:, NR + 1, 1 : W + 1], cst[:, 0, :], ihq1[:, 0:1]
            )
            # h_odd: only the pad cols need to start (and stay) zero
            nc.vector.memset(hb[1][:, :, 0:1], 0.0)
            nc.vector.memset(hb[1][:, :, W + 1 : W + 2], 0.0)
            nc.vector.tensor_scalar_add(cst[:], cst[:], ic_sb[:, 0:1])

            def xpart(ps, x):
                for g in range(4):
                    for tap in range(9):
                        dy, dx = divmod(tap, 3)
                        nc.tensor.matmul(
                            ps[g][:],
                            wx_sb[:, g, tap, :],
                            x[:, dy : dy + NR, dx : dx + W],
                            start=(tap == 0),
                            stop=False,
                        )

            def hpart_int(ps, h):
                # output rows 1..14 <- input buffer rows (1+dy)..(14+dy)
                for g in range(4):
                    for tap in range(9):
                        dy, dx = divmod(tap, 3)
                        nc.tensor.matmul(
                            ps[g][:, ISL, :],
                            wh_sb[:, g, tap, :],
                            h[:, 1 + dy : NR - 1 + dy, dx : dx + W],
                            start=False,
                            stop=False,
                        )

            def hpart_bnd(ps, h):
                # output rows {0,15} <- input buffer rows {dy, 15+dy}
                for g in range(4):
                    for tap in range(9):
                        dy, dx = divmod(tap, 3)
                        nc.tensor.matmul(
                            ps[g][:, BSL, :],
                            wh_sb[:, g, tap, :],
                            h[:, dy : dy + NR : NR - 1, dx : dx + W],
                            start=False,
                            stop=(tap == 8),
                        )

            # ---- prologue: conv x-parts of the first two steps
            ps = {}
            for t0 in range(2):
                ps[t0] = [
                    psum.tile([HID, NR, W], F32, tag=f"ps{g}", name=f"ps{g}_{t0}")
                    for g in range(4)
                ]
                xpart(ps[t0], xt[t0])

            hcur = hb[0]
            for t in range(n_steps):
                if t + 2 < n_steps:
                    xt[t + 2] = xpool.tile([C, BR, BC], BF16, tag="x", name=f"x_{t+2}")
                    nc.sync.dma_start(out=xt[t + 2][:], in_=xs[t + 2])
                hn = hb[(t + 1) % 2]
                p = ps.pop(t)
                hpart_int(p, hcur)
                hpart_bnd(p, hcur)

                # ---- gate activations, ordered so the send-row chain can
                # launch ASAP: fg/ig/gg full, og boundary-only early, og
                # interior later
                fg = work.tile([HID, NR, W], F32, tag="fg", name=f"fg_{t}")
                nc.scalar.activation(fg[:], p[1][:], AFT.Sigmoid, bias=cb_sb[:, 1:2])
                ig = work.tile([HID, NR, W], F32, tag="ig", name=f"ig_{t}")
                nc.scalar.activation(ig[:], p[0][:], AFT.Sigmoid, bias=cb_sb[:, 0:1])
                gg = work.tile([HID, NR, W], F32, tag="gg", name=f"gg_{t}")
                nc.scalar.activation(gg[:], p[3][:], AFT.Tanh, bias=cb_sb[:, 3:4])
                ogb = work.tile([HID, 2, W], F32, tag="ogb", name=f"ogb_{t}")
                nc.scalar.activation(
                    ogb[:], p[2][:, BSL, :], AFT.Sigmoid, bias=cb_sb[:, 2:3]
                )

                # ---- boundary rows first: state update -> send row
                ub = work.tile([HID, 2, W], F32, tag="ub", name=f"ub_{t}")
                nc.vector.tensor_mul(ub[:], fg[:, BSL, :], cst[:, BSL, :])
                vb = work.tile([HID, 2, W], F32, tag="vb", name=f"vb_{t}")
                nc.vector.tensor_mul(vb[:], ig[:, BSL, :], gg[:, BSL, :])
                nc.vector.tensor_add(cst[:, BSL, :], ub[:], vb[:])
                tchb = work.tile([HID, 2, W], F32, tag="tchb", name=f"tchb_{t}")
                nc.scalar.activation(tchb[:], cst[:, BSL, :], AFT.Tanh)
                nc.vector.tensor_mul(
                    hn[:, 1 : NR + 1 : NR - 1, 1 : W + 1], ogb[:], tchb[:]
                )

                if t + 1 < n_steps:
                    # send row: top sends its row 16 (image row 15), bottom row 1
                    tmp = work.tile([HID, W], BF16, tag="tmp", name=f"tmp_{t}")
                    nc.vector.tensor_scalar_mul(tmp[:], hn[:, NR, 1 : W + 1], s0)
                    snd = work.tile([HID, W], BF16, tag="snd", name=f"snd_{t}")
                    nc.vector.scalar_tensor_tensor(
                        snd[:], hn[:, 1, 1 : W + 1], s1, tmp[:],
                        op0=ALU.mult, op1=ALU.add,
                    )
                    agin = dram.tile([HID, W], BF16, tag="agin", name=f"agin_{t}")
                    agout = dram.tile([2 * HID, W], BF16, tag="agout", name=f"agout_{t}")
                    if use_coll:
                        nc.gpsimd.dma_start(out=agin[:], in_=snd[:])
                        nc.gpsimd.collective_compute(
                            "AllGather",
                            ALU.bypass,
                            replica_groups=PAIRS,
                            ins=[agin[:].opt()],
                            outs=[agout[:].opt()],
                        )
                    e01 = work.tile([HID, 2, W], BF16, tag="e01", name=f"e01_{t}")
                    if use_coll:
                        nc.gpsimd.dma_start(
                            out=e01[:], in_=agout[:].rearrange("(j p) w -> p j w", p=HID)
                        )
                    else:
                        nc.vector.memset(e01[:], 0.0)

                # ---- interior rows
                ogi = work.tile([HID, NR - 2, W], F32, tag="ogi", name=f"ogi_{t}")
                nc.scalar.activation(
                    ogi[:], p[2][:, ISL, :], AFT.Sigmoid, bias=cb_sb[:, 2:3]
                )
                ui = work.tile([HID, NR - 2, W], F32, tag="ui", name=f"ui_{t}")
                nc.vector.tensor_mul(ui[:], fg[:, ISL, :], cst[:, ISL, :])
                vi = work.tile([HID, NR - 2, W], F32, tag="vi", name=f"vi_{t}")
                nc.vector.tensor_mul(vi[:], ig[:, ISL, :], gg[:, ISL, :])
                nc.vector.tensor_add(cst[:, ISL, :], ui[:], vi[:])
                tchi = work.tile([HID, NR - 2, W], F32, tag="tchi", name=f"tchi_{t}")
                nc.scalar.activation(tchi[:], cst[:, ISL, :], AFT.Tanh)
                nc.vector.tensor_mul(
                    hn[:, 2:NR, 1 : W + 1], ogi[:], tchi[:]
                )
                nc.vector.tensor_reduce(
                    hsum[:, t : t + 1],
                    hn[:, 1 : NR + 1, 1 : W + 1],
                    axis=AXT.XY,
                    op=ALU.add,
                )
                if use_coll and n_steps >= 4 and t == n_steps - 2:
                    # pair-exchange of the pooled sums for all finished steps;
                    # overlaps with the last scan step (which has no halo
                    # exchange of its own, so the CC queue is free)
                    npre = n_steps - 1
                    hg1i = dram.tile([HID, npre], F32, tag="hg1i", name="hg1i")
                    hg1o = dram.tile([2 * HID, npre], F32, tag="hg1o", name="hg1o")
                    nc.gpsimd.dma_start(out=hg1i[:], in_=hsum[:, 0:npre])
                    nc.gpsimd.collective_compute(
                        "AllGather", ALU.bypass, replica_groups=PAIRS,
                        ins=[hg1i[:].opt()], outs=[hg1o[:].opt()],
                    )
                    eh1 = work.tile([HID, 2, npre], F32, tag="eh1", name="eh1")
                    nc.gpsimd.dma_start(
                        out=eh1[:], in_=hg1o[:].rearrange("(j p) w -> p j w", p=HID)
                    )

                if t + 1 < n_steps:
                    # ghost rows from the exchange (masked per core)
                    nc.vector.tensor_scalar_mul(hn[:, 0, 1 : W + 1], e01[:, 0, :], q0)
                    nc.vector.tensor_scalar_mul(
                        hn[:, NR + 1, 1 : W + 1], e01[:, 1, :], q1
                    )

                if t + 2 < n_steps:
                    ps[t + 2] = [
                        psum.tile([HID, NR, W], F32, tag=f"ps{g}", name=f"ps{g}_{t+2}")
                        for g in range(4)
                    ]
                    xpart(ps[t + 2], xt[t + 2])

                hcur = hn

            # ---- head: pair-reduce the pooled sums, then the two FC layers
            fsum = work.tile([HID, S], F32, tag="fsum", name="fsum")
            if use_coll and n_steps >= 4:
                npre = n_steps - 1
                nc.vector.tensor_add(fsum[:, 0:npre], eh1[:, 0, :], eh1[:, 1, :])
                # last hsum column: ship as bf16 hi/lo pair through the warm
                # halo-shaped AllGather config (the [HID,1] f32 config costs
                # ~12us in-situ; this one is exercised 31x during the scan)
                pk = work.tile([HID, W], BF16, tag="pk", name="pk")
                nc.vector.memset(pk[:], 0.0)
                nc.vector.tensor_copy(pk[:, 0:1], hsum[:, npre:n_steps])
                nc.vector.scalar_tensor_tensor(
                    pk[:, 1:2], pk[:, 0:1], -1.0, hsum[:, npre:n_steps],
                    op0=ALU.mult, op1=ALU.add,
                )
                hg2i = dram.tile([HID, W], BF16, tag="agin", name="hg2i")
                hg2o = dram.tile([2 * HID, W], BF16, tag="agout", name="hg2o")
                nc.gpsimd.dma_start(out=hg2i[:], in_=pk[:])
                nc.gpsimd.collective_compute(
                    "AllGather", ALU.bypass, replica_groups=PAIRS,
                    ins=[hg2i[:].opt()], outs=[hg2o[:].opt()],
                )
                eh2 = work.tile([HID, 2, W], BF16, tag="e01", name="eh2")
                nc.gpsimd.dma_start(
                    out=eh2[:], in_=hg2o[:].rearrange("(j p) w -> p j w", p=HID)
                )
                eh2s = work.tile([HID, 2], F32, tag="eh2s", name="eh2s")
                nc.vector.tensor_add(eh2s[:], eh2[:, :, 0], eh2[:, :, 1])
                nc.vector.tensor_add(
                    fsum[:, npre:n_steps], eh2s[:, 0:1], eh2s[:, 1:2]
                )
                if n_steps < S:
                    nc.vector.memset(fsum[:, n_steps:S], 0.0)
            else:
                nc.vector.tensor_copy(fsum[:], hsum[:])
            pf = psum.tile([C, S], F32, tag="ps0", name="pf")
            nc.tensor.matmul(pf[:], fcw_sb[:], fsum[:], start=True, stop=True)
            feat = work.tile([C, S], F32, tag="feat", name="feat")
            nc.scalar.activation(feat[:], pf[:], AFT.Relu, bias=fcb_sb[:, 0:1])
            ph = psum.tile([2, S], F32, tag="ps1", name="ph")
            nc.tensor.matmul(ph[:], fhw_sb[:], feat[:], start=True, stop=True)
            oa = work.tile([2, S], F32, tag="oa", name="oa")
            nc.scalar.activation(oa[:], ph[:], AFT.Identity, bias=fhb_sb[:, 0:1])
            nc.sync.dma_start(out=out, in_=oa[:])

    nc.compile()
    return nc


def _prep_in_maps(x, conv_w, conv_b, init_h, init_c, fc_w, fc_b, fco_w, fco_b, fca_w, fca_b):
    f = np.float32
    cw = np.asarray(conv_w, f).reshape(4, HID, C + HID, 3, 3)  # [g, m, kin, dy, dx]
    # lhsT layout [k, g, tap, m]
    wx = np.ascontiguousarray(
        cw[:, :, :C].transpose(2, 0, 3, 4, 1).reshape(C, 4, 9, HID)
    ).astype(NP_BF16)
    wh = np.ascontiguousarray(
        cw[:, :, C:].transpose(2, 0, 3, 4, 1).reshape(HID, 4, 9, HID)
    ).astype(NP_BF16)
    cb = np.ascontiguousarray(np.asarray(conv_b, f).reshape(4, HID).T)  # [HID, 4]
    ih = np.asarray(init_h, f).reshape(HID, 1)
    ic = np.asarray(init_c, f).reshape(HID, 1)
    # fold the 1/(H*W) spatial mean into fc_w;  lhsT = fc_w.T
    fcw = np.ascontiguousarray(np.asarray(fc_w, f).T / f(H * W))  # [HID, C]
    fcb = np.asarray(fc_b, f).reshape(C, 1)
    fhw = np.ascontiguousarray(
        np.stack([np.asarray(fco_w, f)[0], np.asarray(fca_w, f)[0]], axis=1)
    )  # [C, 2]
    fhb = np.array([[np.asarray(fco_b, f)[0]], [np.asarray(fca_b, f)[0]]], f)  # [2, 1]

    x = np.asarray(x, f)
    in_maps = []
    for b in range(B):
        for half in range(2):
            xs = np.zeros((S, C, BR, BC), f)
            if half == 0:  # top: image rows -1..16, row -1 is zero padding
                xs[:, :, 1:BR, 1 : W + 1] = x[b][:, :, 0 : NR + 1, :]
                m = [1.0, 0.0, 0.0, 1.0]
            else:  # bottom: image rows 15..32, row 32 is zero padding
                xs[:, :, 0 : BR - 1, 1 : W + 1] = x[b][:, :, NR - 1 : H, :]
                m = [0.0, 1.0, 1.0, 0.0]
            msk = np.ascontiguousarray(np.broadcast_to(np.array(m, f), (128, 4)))
            in_maps.append(
                dict(
                    xs=xs.astype(NP_BF16), wx=wx, wh=wh, cb=cb, ih=ih, ic=ic,
                    fcw=fcw, fcb=fcb, fhw=fhw, fhb=fhb, msk=msk,
                )
            )
    return in_maps


def _numpy_ref(x, conv_w, conv_b, init_h, init_c, fc_w, fc_b, fco_w, fco_b, fca_w, fca_b):
    f = np.float32
    x = np.asarray(x, f)
    b_, s_, c_, h_, w_ = x.shape
    hid = init_h.shape[0]
    hcur = np.broadcast_to(np.asarray(init_h, f)[None, :, None, None], (b_, hid, h_, w_)).copy()
    cst = np.broadcast_to(np.asarray(init_c, f)[None, :, None, None], (b_, hid, h_, w_)).copy()
    wxy = np.asarray(conv_w, f)  # [4h, c+hid, 3, 3]
    feats = np.zeros((b_, s_, hid), f)

    def conv(z):
        zp = np.pad(z, ((0, 0), (0, 0), (1, 1), (1, 1)))
        out = np.zeros((b_, 4 * hid, h_, w_), f)
        for dy in range(3):
            for dx in range(3):
                out += np.einsum(
                    "ok,bkhw->bohw", wxy[:, :, dy, dx],
                    zp[:, :, dy : dy + h_, dx : dx + w_],
                    optimize=True,
                )
        return out + np.asarray(conv_b, f)[None, :, None, None]

    def sig(v):
        return 1.0 / (1.0 + np.exp(-v))

    for t in range(s_):
        z = np.concatenate([x[:, t], hcur], axis=1)
        g = conv(z)
        i, fo, o, gg = np.split(g, 4, axis=1)
        cst = sig(fo) * cst + sig(i) * np.tanh(gg)
        hcur = sig(o) * np.tanh(cst)
        feats[:, t] = hcur.mean(axis=(2, 3))
    feat = np.maximum(feats @ np.asarray(fc_w, f).T + np.asarray(fc_b, f), 0.0)
    offset = feat @ np.asarray(fco_w, f).T + np.asarray(fco_b, f)
    angle = feat @ np.asarray(fca_w, f).T + np.asarray(fca_b, f)
    return offset.astype(f), angle.astype(f)


def kernel(x, conv_w, conv_b, init_h, init_c, fc_w, fc_b, fco_w, fco_b, fca_w, fca_b,
           _return_bass_results=False, _trace=False, _use_coll=True):
    args = (x, conv_w, conv_b, init_h, init_c, fc_w, fc_b, fco_w, fco_b, fca_w, fca_b)
    try:
        key = ("nc", _use_coll)
        if key not in _cache:
            _cache[key] = _build(_use_coll)
        nc = _cache[key]
        in_maps = _prep_in_maps(*args)
        res = run_bass_kernel_spmd(nc, in_maps, list(range(8)), trace=_trace)
        offset = np.zeros((B, S, 1), np.float32)
        angle = np.zeros((B, S, 1), np.float32)
        for b in range(B):
            o = res.results[2 * b]["out"]
            offset[b, :, 0] = o[0]
            angle[b, :, 0] = o[1]
    except Exception:
        if _return_bass_results:
            raise
        o, a = _numpy_ref(*args)
        return o, a
    if _return_bass_results:
        return (offset, angle), res
    return (offset, angle)
